# revision 1
# baseline (speedup 1.0000x reference)
"""ChebConv (K=4) Trainium2 Bass kernel.

Problem (hardcoded): B=16 graphs, N=2048 nodes, F=64 feats, K=4, out_dim=128.
  L = D A0 D  (A0 = A with zeroed diag, D = diag(1/(eps+sqrt(rowsum(A0)))))
  T0 = X; T1 = L X; T_t = 2 L T_{t-1} - T_{t-2}
  out = relu(concat(T0..T3) @ kernel + bias)

Sharding: batch across 8 cores, 2 graphs per core. Each core gets the full
kernel/bias (replicated) and its A/X slice; host concatenates the outputs.

Device algorithm (per core, graphs g=0,1):
  Z_t := d * T_t  (rowwise). Then
    Z0      = d*X
    Z1      = d^2 * (A0 @ Z0)
    Z_{t+1} = 2 d^2 * (A0 @ Z_t) - Z_{t-1}
    out     = relu( (1/d) * (sum_t Z_t @ K_t) + bias )
  The (1/d) row scale commutes with the right-multiply, and is folded into
  the Z^T tiles used by the projection (via a diag(e) matmul).

  A arrives f32 in HBM; the SWDGE DMA casts it to bf16 on the fly. Row sums
  are the accum_out of a DVE sweep. A^T (needed because the PE contracts
  over the partition axis) is built by identity-matmuls on the PE, 128x128
  tiles, drained PSUM->SBUF on ACT/DVE. The two graphs' Chebyshev matmuls
  are column-tiled into the two halves of the PE array so they run
  concurrently.
"""

import numpy as np

P = 128          # partitions
N = 2048         # nodes per graph
F = 64           # input features
KORD = 4         # Chebyshev order
OUT = 128        # output features
GP = 2           # graphs per core
NT = N // P      # 16 node chunks
NS = N // 512    # 4 moving strips
NCORES = 8

_cached = {}


def _build_nc():
    import ml_dtypes
    import concourse.bacc as bacc
    import concourse.mybir as mybir
    from concourse.tile import TileContext

    f32 = mybir.dt.float32
    bf16 = mybir.dt.bfloat16
    Alu = mybir.AluOpType
    Act = mybir.ActivationFunctionType

    nc = bacc.Bacc("TRN2", target_bir_lowering=False)

    a_in = nc.dram_tensor("a", [GP, N, N], f32, kind="ExternalInput")
    x_in = nc.dram_tensor("x", [GP, N, F], f32, kind="ExternalInput")
    wk_in = nc.dram_tensor("wk", [KORD * F, OUT], f32, kind="ExternalInput")
    bias_in = nc.dram_tensor("bias", [OUT], f32, kind="ExternalInput")
    o_out = nc.dram_tensor("out", [GP, N, OUT], f32, kind="ExternalOutput")

    ident_np = np.eye(P, dtype=ml_dtypes.bfloat16)
    ident_dram = nc.inline_tensor(ident_np, name="identbf")

    with TileContext(nc) as tc, tc.tile_pool(name="const", bufs=1) as const, \
         tc.tile_pool(name="big", bufs=1) as big, \
         tc.tile_pool(name="astage", bufs=6) as astage, \
         tc.tile_pool(name="small", bufs=1) as small, \
         tc.tile_pool(name="xstage", bufs=6) as xstage, \
         tc.tile_pool(name="wt", bufs=3) as wtpool, \
         tc.tile_pool(name="dgepool", bufs=6) as dgepool, \
         tc.tile_pool(name="outs", bufs=2) as outs, \
         tc.tile_pool(name="ps_iter", bufs=4, space="PSUM") as ps_iter, \
         tc.tile_pool(name="ps_tr", bufs=3, space="PSUM") as ps_tr, \
         tc.tile_pool(name="ps_z", bufs=1, space="PSUM") as ps_z:

        # ---- constants -------------------------------------------------
        ident = const.tile([P, P], bf16)
        nc.sync.dma_start(out=ident, in_=ident_dram[:, :])
        # mask = 1 - I  (bf16)
        mask = const.tile([P, P], bf16)
        nc.vector.tensor_scalar(mask, ident, -1.0, 1.0, Alu.mult, Alu.add)
        # kernel: [256,128] f32 -> two bf16 tiles [128,128] (t-pairs 01, 23)
        kab = const.tile([P, OUT], bf16)
        kcd = const.tile([P, OUT], bf16)
        kstage = astage.tile([P, 2 * OUT], f32, name="kstage")
        nc.sync.dma_start(out=kstage[:, 0:OUT], in_=wk_in[0:P, :])
        nc.sync.dma_start(out=kstage[:, OUT : 2 * OUT], in_=wk_in[P : 2 * P, :])
        nc.vector.tensor_copy(kab, kstage[:, 0:OUT])
        nc.vector.tensor_copy(kcd, kstage[:, OUT : 2 * OUT])
        # bias row [1,128] bf16 + ones row [1,128] bf16
        bias_row = const.tile([1, OUT], bf16)
        bias_f32 = const.tile([1, OUT], f32)
        nc.sync.dma_start(out=bias_f32, in_=bias_in[None, :])
        nc.vector.tensor_copy(bias_row, bias_f32)
        ones_row = const.tile([1, P], bf16)
        nc.vector.memset(ones_row, 1.0)

        # ---- persistent SBUF state ------------------------------------
        # A^T per graph: [:, q, :] is j-tile q (j = 128q+p), free = node i
        at = [big.tile([P, NT, N], bf16, name=f"at{g}") for g in range(GP)]
        # Z_t, joint layout: [:, q, 0:64] = graph0, [:, q, 64:128] = graph1
        zj = [big.tile([P, NT, 2 * F], bf16, name=f"zj{t}") for t in range(KORD)]
        # e-scaled Z^T pairs per graph: rows 0:64 = even t, 64:128 = odd t
        ztab = [big.tile([P, N], bf16, name=f"ztab{g}") for g in range(GP)]
        ztcd = [big.tile([P, N], bf16, name=f"ztcd{g}") for g in range(GP)]
        # joint row stats [128, NT, GP] f32 (cols: node chunk x graph)
        rs = small.tile([P, NT, GP], f32, name="rs")
        dd = small.tile([P, NT, GP], f32, name="dd")      # d/2
        d2s = small.tile([P, NT, GP], f32, name="d2s")    # 2 d^2
        evec = small.tile([P, NT, GP], f32, name="evec")  # 1/d
        junk = small.tile([P, N], bf16, name="junk")

        # ---- phase A (fused): load, cast, rowsum, transpose, d-chain,
        # Z0, and iteration-1 matmuls chasing the incoming chunks -------
        # psum tiles for iteration 1, one [128,512] bank per node strip;
        # iteration 2/3 strip tiles later recycle these pool slots.
        psum1 = []
        for c in range(NT):
            for g in range(GP):
                ach = astage.tile([P, N], bf16, name="ach", tag="ach")
                nc.gpsimd.dma_start(out=ach, in_=a_in[g, c * P : (c + 1) * P, :])
                # zero the diagonal block in place (scalar_tensor_tensor:
                # TT-struct instructions only carry one HW sync-wait slot)
                nc.vector.scalar_tensor_tensor(
                    ach[:, c * P : (c + 1) * P],
                    ach[:, c * P : (c + 1) * P], 1.0, mask,
                    Alu.mult, Alu.mult,
                )
                # row sums (of the diag-zeroed rows) as accum_out
                nc.vector.tensor_scalar(
                    junk, ach, 1.0, None, Alu.mult, Alu.add,
                    accum_out=rs[:, c, g : g + 1],
                )
                # transpose the 16 128x128 blocks via PE; drain 4 at a time
                for k in range(NS):
                    tr = ps_tr.tile([P, 512], mybir.dt.float32, name="tr", tag="tr")
                    for j in range(4):
                        q = 4 * k + j
                        nc.tensor.matmul(
                            tr[:, j * P : (j + 1) * P],
                            lhsT=ach[:, q * P : (q + 1) * P],
                            rhs=ident,
                            start=(j == 0), stop=(j == 3),
                        )
                    dst = at[g][:, 4 * k : 4 * k + 4, c * P : (c + 1) * P]
                    if k == 0:
                        nc.vector.tensor_copy(out=dst, in_=tr)
                    else:
                        nc.scalar.copy(out=dst, in_=tr)
            # d chain for chunk c, both graphs at once ([128, 2] ops).
            # Newton-refined sqrt: t = sqrt(rs); w = rs/t + t (= 2 sqrt(rs))
            tch = small.tile([P, GP], f32, name="tch", tag="tch")
            uch = small.tile([P, GP], f32, name="uch", tag="uch")
            wch = small.tile([P, GP], f32, name="wch", tag="wch")
            rsc = rs[:, c, :]
            nc.scalar.activation(tch, rsc, Act.Sqrt)
            nc.vector.reciprocal(uch, tch)
            nc.vector.scalar_tensor_tensor(uch, uch, 1.0, rsc, Alu.mult, Alu.mult)
            nc.vector.scalar_tensor_tensor(wch, uch, 1.0, tch, Alu.mult, Alu.add)
            # dd = 1/w = d/2 ; e = w/2 = 1/d ; d2s = 8*dd^2 = 2 d^2
            nc.vector.reciprocal(dd[:, c, :], wch)
            nc.vector.tensor_scalar_mul(evec[:, c, :], wch, 0.5)
            nc.vector.scalar_tensor_tensor(
                d2s[:, c, :], dd[:, c, :], 8.0, dd[:, c, :], Alu.mult, Alu.mult
            )
            # Z0 = d*X = (X*dd)*2
            for g in range(GP):
                xst = xstage.tile([P, F], f32, name="xst", tag="xst")
                nc.sync.dma_start(out=xst, in_=x_in[g, c * P : (c + 1) * P, :])
                nc.vector.tensor_scalar(
                    zj[0][:, c, g * F : (g + 1) * F], xst,
                    dd[:, c, g : g + 1], 2.0, Alu.mult, Alu.mult,
                )
            # iteration-1 matmuls now runnable:
            #  - strips whose A^T columns completed earlier get their q=c term
            #  - if chunk c completes strip c//4, catch that strip up (q<=c)
            def it1_mm(q, s):
                for g in range(GP):
                    nc.tensor.matmul(
                        psum1[s][g * F : (g + 1) * F, :],
                        lhsT=zj[0][:, q, g * F : (g + 1) * F],
                        rhs=at[g][:, q, s * 512 : (s + 1) * 512],
                        start=(q == 0), stop=(q == NT - 1),
                        tile_position=(0, g * F),
                        skip_group_check=True,
                    )
            for s in range(c // 4):
                it1_mm(c, s)
            if c % 4 == 3:
                s = c // 4
                psum1.append(
                    ps_iter.tile([P, 512], mybir.dt.float32,
                                 name=f"psum1_{s}", tag="psum")
                )
                for q in range(c + 1):
                    it1_mm(q, s)

        # ---- phases C/D/E: strip-pipelined iterations -----------------
        # per iteration t: for each 512-wide node strip s, accumulate the
        # two graphs' (A0@Z)^T halves into one psum bank (column-tiled PE),
        # drain to bf16, re-transpose back to node-major, and combine into
        # Z_t. Z'^T pair builds (phase D) are emitted right after the
        # iteration that completes the pair; projection/relu/store follow
        # the last pair, per strip. Tile's dependency tracking turns the
        # emission order into a pipeline.

        def build_ztpair(ta, tb, ztdst, s):
            # e-scaled transposed copies of Z_ta / Z_tb for strip s
            for g in range(GP):
                psz = ps_z.tile([P, 512], mybir.dt.float32, name="psz", tag="psz")
                for j in range(4):
                    c = 4 * s + j
                    dge = dgepool.tile([P, P], bf16, name="dge", tag="dge")
                    nc.vector.tensor_scalar_mul(dge, ident, evec[:, c, g : g + 1])
                    for row0, t in ((0, ta), (F, tb)):
                        nc.tensor.matmul(
                            psz[row0 : row0 + F, j * P : (j + 1) * P],
                            lhsT=zj[t][:, c, g * F : (g + 1) * F],
                            rhs=dge,
                            start=(j == 0), stop=(j == 3),
                            tile_position=(0, row0),
                            skip_group_check=True,
                        )
                nc.scalar.copy(out=ztdst[g][:, s * 512 : (s + 1) * 512], in_=psz)

        def project_strip(g, s):
            pso = ps_z.tile([P, 512], mybir.dt.float32, name="pso", tag="psz")
            for j in range(4):
                c = 4 * s + j
                sl = pso[:, j * OUT : (j + 1) * OUT]
                nc.tensor.matmul(
                    sl, lhsT=ztab[g][:, c * P : (c + 1) * P], rhs=kab,
                    start=(j == 0), stop=False,
                )
                nc.tensor.matmul(
                    sl, lhsT=ztcd[g][:, c * P : (c + 1) * P], rhs=kcd,
                    start=False, stop=False,
                )
                nc.tensor.matmul(
                    sl, lhsT=ones_row, rhs=bias_row,
                    start=False, stop=(j == 3),
                )
            ot = outs.tile([P, 4, OUT], f32, name="ot", tag="ot")
            for j in range(4):
                # relu on DVE (tensor_scalar max) -- faster than ACT and DVE
                # is idle in the tail
                nc.vector.tensor_scalar_max(
                    ot[:, j, :], pso[:, j * OUT : (j + 1) * OUT], 0.0
                )
            nc.sync.dma_start(
                out=o_out[g, s * 512 : (s + 1) * 512, :].rearrange(
                    "(j p) o -> p j o", p=P
                ),
                in_=ot,
            )

        for t in range(1, KORD):
            for s in range(NS):
                if t == 1:
                    psum = psum1[s]  # already accumulated during the load
                else:
                    psum = ps_iter.tile(
                        [P, 512], mybir.dt.float32, name="psum", tag="psum"
                    )
                    for q in range(NT):
                        for g in range(GP):
                            nc.tensor.matmul(
                                psum[g * F : (g + 1) * F, :],
                                lhsT=zj[t - 1][:, q, g * F : (g + 1) * F],
                                rhs=at[g][:, q, s * 512 : (s + 1) * 512],
                                start=(q == 0), stop=(q == NT - 1),
                                tile_position=(0, g * F),
                                skip_group_check=True,
                            )
                # drain this strip of (A0@Z)^T to bf16 SBUF
                wt = wtpool.tile([P, 512], bf16, name="wt", tag="wt")
                if s % 2 == 0:
                    nc.scalar.copy(out=wt, in_=psum)
                else:
                    nc.vector.tensor_copy(out=wt, in_=psum)
                # re-transpose back to node-major and combine into Z_t
                tr = ps_tr.tile([P, 512], mybir.dt.float32, name="tr2", tag="tr")
                for j in range(4):
                    nc.tensor.matmul(
                        tr[:, j * P : (j + 1) * P],
                        lhsT=wt[:, j * P : (j + 1) * P],
                        rhs=ident,
                        start=(j == 0), stop=(j == 3),
                    )
                for j in range(4):
                    c = 4 * s + j
                    for g in range(GP):
                        w_ng = tr[:, j * P + g * F : j * P + (g + 1) * F]
                        zdst = zj[t][:, c, g * F : (g + 1) * F]
                        if t == 1:
                            # Z1 = d^2 * W = (W * d2s) * 0.5
                            nc.vector.tensor_scalar(
                                zdst, w_ng, d2s[:, c, g : g + 1], 0.5,
                                Alu.mult, Alu.mult,
                            )
                        else:
                            # Z_t = (W * d2s) - Z_{t-2}
                            nc.vector.scalar_tensor_tensor(
                                zdst, w_ng, d2s[:, c, g : g + 1],
                                zj[t - 2][:, c, g * F : (g + 1) * F],
                                Alu.mult, Alu.subtract,
                            )
                if t == 1:
                    build_ztpair(0, 1, ztab, s)
                elif t == 3:
                    build_ztpair(2, 3, ztcd, s)
                    for g in range(GP):
                        project_strip(g, s)

    nc.finalize()
    return nc


def _get_nc():
    if "nc" not in _cached:
        _cached["nc"] = _build_nc()
    return _cached["nc"]


def kernel(X, A, kernel, bias):
    from concourse.bass_utils import run_bass_kernel_spmd

    nc = _get_nc()
    wk = np.ascontiguousarray(np.asarray(kernel, dtype=np.float32))
    bs = np.ascontiguousarray(np.asarray(bias, dtype=np.float32))
    A = np.asarray(A, dtype=np.float32)
    X = np.asarray(X, dtype=np.float32)
    in_maps = [
        {
            "a": np.ascontiguousarray(A[GP * c : GP * (c + 1)]),
            "x": np.ascontiguousarray(X[GP * c : GP * (c + 1)]),
            "wk": wk,
            "bias": bs,
        }
        for c in range(NCORES)
    ]
    res = run_bass_kernel_spmd(nc, in_maps, core_ids=list(range(NCORES)))
    return np.concatenate([r["out"] for r in res.results], axis=0)



# revision 8
# speedup vs baseline: 1.2224x; 1.2224x over previous
"""ChebConv (K=4) Trainium2 Bass kernel — node-major mapping.

Problem (hardcoded): B=16 graphs, N=2048 nodes, F=64 feats, K=4, out_dim=128.
  L = D A0 D  (A0 = A with zeroed diag, D = diag(1/(eps+sqrt(rowsum(A0)))))
  T0 = X; T1 = L X; T_t = 2 L T_{t-1} - T_{t-2}
  out = relu(concat(T0..T3) @ kernel + bias)

Sharding: batch across 8 cores, 2 graphs per core; host concatenates.

Device algorithm (per core, graphs g=0,1), with V_t := 2d * T_t:
    V0      = 2d*X
    W_t     = A0 @ V_t          (pure bf16 matmul, A^T blocks as weights)
    V1      = d^2 * W0
    V_{t+1} = 2d^2 * W_t - V_{t-1}
    out     = relu( sum_t (V_t/(2d)) @ K_t + bias )

Key layout choice: the Chebyshev matmuls run NODE-major — each output tile
is [128 nodes x 64 feats] with an A^T 128x128 block as the stationary
(lhsT) operand and V as the moving rhs. Outputs are full 128 partitions
wide, so the PE streams half the rows of the feature-major alternative,
and W lands node-major so the V update is a single elementwise op (no
re-transpose). Row sums ride the PE too (ones-column matmuls against the
A^T blocks), freeing the DVE. A arrives f32 in HBM; the SWDGE DMA casts
to bf16 on the fly (2 node-chunks per DMA). A^T is built by identity
matmuls, drained PSUM->SBUF round-robin over ACT/DVE/Pool. The final
projection reads e-scaled Z^T tiles built by diag(e) matmuls; bias is
added with a rank-1 ones x bias matmul and relu rides the PSUM drain.
"""

import numpy as np

P = 128          # partitions
N = 2048         # nodes per graph
F = 64           # input features
OUT = 128        # output features
GP = 2           # graphs per core
NT = N // P      # 16 node chunks
CH = 2           # node chunks per A-load DMA
NCORES = 8

_cached = {}


def _build_nc():
    import ml_dtypes
    import concourse.bacc as bacc
    import concourse.mybir as mybir
    from concourse.tile import TileContext

    f32 = mybir.dt.float32
    bf16 = mybir.dt.bfloat16
    Alu = mybir.AluOpType
    Act = mybir.ActivationFunctionType

    nc = bacc.Bacc("TRN2", target_bir_lowering=False)

    a_in = nc.dram_tensor("a", [GP, N, N], f32, kind="ExternalInput")
    x_in = nc.dram_tensor("x", [GP, N, F], f32, kind="ExternalInput")
    wk_in = nc.dram_tensor("wk", [2 * P, OUT], f32, kind="ExternalInput")
    bias_in = nc.dram_tensor("bias", [OUT], f32, kind="ExternalInput")
    o_out = nc.dram_tensor("out", [GP, N, OUT], f32, kind="ExternalOutput")

    ident_np = np.eye(P, dtype=ml_dtypes.bfloat16)
    ident_dram = nc.inline_tensor(ident_np, name="identbf")

    with TileContext(nc) as tc, \
         tc.tile_pool(name="const", bufs=1) as const, \
         tc.tile_pool(name="big", bufs=1) as big, \
         tc.tile_pool(name="astage", bufs=3) as astage, \
         tc.tile_pool(name="dch", bufs=2) as dch, \
         tc.tile_pool(name="dgep", bufs=4) as dgep, \
         tc.tile_pool(name="outs", bufs=2) as outs, \
         tc.tile_pool(name="ps_tr", bufs=2, space="PSUM") as ps_tr, \
         tc.tile_pool(name="ps_it", bufs=3, space="PSUM") as ps_it, \
         tc.tile_pool(name="ps_rs", bufs=1, space="PSUM") as ps_rs, \
         tc.tile_pool(name="ps_z", bufs=2, space="PSUM") as ps_z:

        # ---- constants -------------------------------------------------
        ident = const.tile([P, P], bf16)
        nc.sync.dma_start(out=ident, in_=ident_dram[:, :])
        mask = const.tile([P, P], bf16)   # 1 - I
        nc.vector.tensor_scalar(mask, ident, -1.0, 1.0, Alu.mult, Alu.add)
        kab = const.tile([P, OUT], bf16)
        kcd = const.tile([P, OUT], bf16)
        kstage = const.tile([P, 2 * OUT], f32)
        nc.sync.dma_start(out=kstage[:, 0:OUT], in_=wk_in[0:P, :])
        nc.sync.dma_start(out=kstage[:, OUT : 2 * OUT], in_=wk_in[P : 2 * P, :])
        nc.vector.tensor_copy(kab, kstage[:, 0:OUT])
        nc.vector.tensor_copy(kcd, kstage[:, OUT : 2 * OUT])
        bias_f32 = const.tile([1, OUT], f32)
        nc.sync.dma_start(out=bias_f32, in_=bias_in[None, :])
        bias4 = const.tile([1, 4 * OUT], bf16)   # bias tiled 4x along free
        for j in range(4):
            nc.vector.tensor_copy(bias4[:, j * OUT : (j + 1) * OUT], bias_f32)
        ones_row = const.tile([1, P], bf16)
        nc.vector.memset(ones_row, 1.0)
        ones_col = const.tile([P, 1], bf16)
        nc.vector.memset(ones_col, 1.0)

        # ---- persistent SBUF state ------------------------------------
        # A^T per graph: [:, q, :] is j-tile q (j = 128q+p), free = node i
        at = [big.tile([P, NT, N], bf16, name=f"at{g}") for g in range(GP)]
        # V pairs, node-major: [:, c, 0:64] = V_t even, [:, c, 64:128] = odd
        zp01 = [big.tile([P, NT, 2 * F], bf16, name=f"zp01_{g}") for g in range(GP)]
        zp23 = [big.tile([P, NT, 2 * F], bf16, name=f"zp23_{g}") for g in range(GP)]
        # e-scaled Z^T pairs for the projection (feature-major)
        ztab = [big.tile([P, N], bf16, name=f"ztab{g}") for g in range(GP)]
        ztcd = [big.tile([P, N], bf16, name=f"ztcd{g}") for g in range(GP)]
        xst = [big.tile([P, NT, F], f32, name=f"xst{g}") for g in range(GP)]
        dsq = big.tile([P, NT, GP], f32, name="dsq")   # d^2
        d2s = big.tile([P, NT, GP], f32, name="d2s")   # 2 d^2
        eh = big.tile([P, NT, GP], f32, name="eh")     # 1/(2d)

        # all 32 per-(graph,chunk) rowsum accumulators live in one bank
        rs_ps = ps_rs.tile([P, 512], f32, name="rsps")

        drain_rr = [0]

        def drain(dst, src):
            k = drain_rr[0] % 2
            drain_rr[0] += 1
            if k == 1:
                nc.vector.tensor_copy(out=dst, in_=src)
            else:
                nc.scalar.copy(out=dst, in_=src)

        def reg(w, i):
            return w[i // 8][:, (i % 8) * F : (i % 8 + 1) * F]

        # PSUM pending-zero is tracked per bank (2KB per partition): only the
        # FIRST write to a bank may carry start=True; later writes to
        # still-pending bytes replace, to cleared bytes accumulate.
        rs_first = [True]

        def emit_rowsum(g, c):
            col = g * NT + c
            for q in range(NT):
                nc.tensor.matmul(
                    rs_ps[:, col : col + 1],
                    lhsT=at[g][:, q, c * P : (c + 1) * P],
                    rhs=ones_col,
                    start=rs_first[0], stop=(q == NT - 1),
                    skip_group_check=True,
                )
                rs_first[0] = False

        it_ps = {}

        # ---- load phase: stream A, cast, transpose, rowsum, d, V0, it1 -
        def load_phase(g):
            nc.sync.dma_start(
                out=xst[g], in_=x_in[g].rearrange("(c p) f -> p c f", p=P)
            )
            w1 = [ps_it.tile([P, 512], f32, name=f"w1_{g}{h}", tag="it")
                  for h in range(2)]
            it_ps[g] = w1
            cnt = [0] * NT
            bank_first = [True, True]
            pend_rs = []

            def emit_it1(i, q):
                nc.tensor.matmul(
                    reg(w1, i),
                    lhsT=at[g][:, q, i * P : (i + 1) * P],
                    rhs=zp01[g][:, q, 0:F],
                    start=bank_first[i // 8], stop=(cnt[i] == NT - 1),
                    skip_group_check=True,
                )
                bank_first[i // 8] = False
                cnt[i] += 1

            for blk in range(NT // CH):
                ach = astage.tile([P, CH, N], bf16, name="ach", tag="ach")
                nc.gpsimd.dma_start(
                    out=ach,
                    in_=a_in[g, blk * CH * P : (blk + 1) * CH * P, :].rearrange(
                        "(k p) j -> p k j", p=P
                    ),
                )
                for k in range(CH):
                    c = blk * CH + k
                    achc = ach[:, k, :]
                    # zero the diagonal block in place
                    nc.vector.scalar_tensor_tensor(
                        achc[:, c * P : (c + 1) * P],
                        achc[:, c * P : (c + 1) * P], 1.0, mask,
                        Alu.mult, Alu.mult,
                    )
                    # transpose 16 blocks; drain 4 at a time over ACT/DVE/Pool
                    for s4 in range(4):
                        tr = ps_tr.tile([P, 512], f32, name="tr", tag="tr")
                        for j in range(4):
                            q = 4 * s4 + j
                            nc.tensor.matmul(
                                tr[:, j * P : (j + 1) * P],
                                lhsT=achc[:, q * P : (q + 1) * P],
                                rhs=ident,
                                start=(j == 0), stop=(j == 3),
                            )
                        drain(at[g][:, 4 * s4 : 4 * s4 + 4, c * P : (c + 1) * P], tr)
                    # rowsums ride the PE, one chunk behind the transposes
                    pend_rs.append(c)
                    if len(pend_rs) > 1:
                        emit_rowsum(g, pend_rs.pop(0))
                if blk % 2 == 1:
                    while pend_rs:
                        emit_rowsum(g, pend_rs.pop(0))
                    grp = blk // 2
                    lo, hi = 4 * grp, 4 * grp + 4
                    # d chain for 4 chunks at once (Newton-refined sqrt)
                    tch = dch.tile([P, 4], f32, name="tch", tag="tch")
                    uch = dch.tile([P, 4], f32, name="uch", tag="uch")
                    wch = dch.tile([P, 4], f32, name="wch", tag="wch")
                    rc = rs_ps[:, g * NT + lo : g * NT + hi]
                    nc.scalar.activation(tch, rc, Act.Sqrt)
                    nc.vector.reciprocal(uch, tch)
                    nc.vector.scalar_tensor_tensor(uch, uch, 1.0, rc, Alu.mult, Alu.mult)
                    nc.vector.scalar_tensor_tensor(wch, uch, 1.0, tch, Alu.mult, Alu.add)
                    nc.vector.reciprocal(uch, wch)   # = d/2
                    nc.vector.scalar_tensor_tensor(
                        dsq[:, lo:hi, g], uch, 4.0, uch, Alu.mult, Alu.mult)
                    nc.vector.scalar_tensor_tensor(
                        d2s[:, lo:hi, g], uch, 8.0, uch, Alu.mult, Alu.mult)
                    nc.vector.tensor_scalar_mul(eh[:, lo:hi, g], wch, 0.25)
                    # V0 = 2d*X = (X * (d/2)) * 4
                    for c in range(lo, hi):
                        nc.vector.tensor_scalar(
                            zp01[g][:, c, 0:F], xst[g][:, c, :],
                            uch[:, c - lo : c - lo + 1], 4.0,
                            Alu.mult, Alu.mult,
                        )
                    # iteration-1 terms now runnable
                    for i in range(lo, hi):
                        for q in range(0, lo):
                            emit_it1(i, q)
                    for q in range(lo, hi):
                        for i in range(0, hi):
                            emit_it1(i, q)

        # ---- Z^T pair build (projection operand) ----------------------
        def ztbuild(g, zp, ztdst):
            for half in range(4):
                psz = ps_z.tile([P, 512], f32, name="psz", tag="psz")
                for j in range(4):
                    c = 4 * half + j
                    dge = dgep.tile([P, P], bf16, name="dge", tag="dge")
                    nc.vector.tensor_scalar_mul(dge, ident, eh[:, c, g : g + 1])
                    nc.tensor.matmul(
                        psz[:, j * P : (j + 1) * P],
                        lhsT=zp[g][:, c, :], rhs=dge,
                        start=(j == 0), stop=(j == 3), skip_group_check=True,
                    )
                dst = ztdst[g][:, half * 512 : (half + 1) * 512]
                if half % 2 == 0:
                    nc.scalar.copy(out=dst, in_=psz)
                else:
                    nc.vector.tensor_copy(out=dst, in_=psz)

        # ---- post-load pipeline per graph ------------------------------
        def rest_phase(g):
            w1 = it_ps[g]
            # V1 = d^2 * W0
            for c in range(NT):
                nc.vector.tensor_scalar(
                    zp01[g][:, c, F : 2 * F], reg(w1, c),
                    dsq[:, c, g : g + 1], 1.0, Alu.mult, Alu.mult,
                )
            ztbuild(g, zp01, ztab)
            # iterations 2 and 3
            for t in (2, 3):
                w = [ps_it.tile([P, 512], f32, name=f"w{t}_{g}{h}", tag="it")
                     for h in range(2)]
                rhs_t = zp01[g] if t == 2 else zp23[g]
                rcol = slice(F, 2 * F) if t == 2 else slice(0, F)
                for i in range(NT):
                    for q in range(NT):
                        nc.tensor.matmul(
                            reg(w, i),
                            lhsT=at[g][:, q, i * P : (i + 1) * P],
                            rhs=rhs_t[:, q, rcol],
                            start=(i % 8 == 0 and q == 0),
                            stop=(q == NT - 1),
                            skip_group_check=True,
                        )
                # V_{t+1} = 2d^2 * W_t - V_{t-1}
                dst = slice(0, F) if t == 2 else slice(F, 2 * F)
                prev = slice(0, F) if t == 2 else slice(F, 2 * F)
                for c in range(NT):
                    nc.vector.scalar_tensor_tensor(
                        zp23[g][:, c, dst], reg(w, c),
                        d2s[:, c, g : g + 1], zp01[g][:, c, prev],
                        Alu.mult, Alu.subtract,
                    )
            ztbuild(g, zp23, ztcd)
            # projection + bias + relu + store, one 512-node bank at a time
            for half in range(4):
                pso = ps_z.tile([P, 512], f32, name="pso", tag="psz")
                for j in range(4):
                    c = 4 * half + j
                    r = pso[:, j * OUT : (j + 1) * OUT]
                    nc.tensor.matmul(
                        r, lhsT=ztab[g][:, c * P : (c + 1) * P], rhs=kab,
                        start=(j == 0), stop=False, skip_group_check=True,
                    )
                    nc.tensor.matmul(
                        r, lhsT=ztcd[g][:, c * P : (c + 1) * P], rhs=kcd,
                        start=False, stop=False, skip_group_check=True,
                    )
                nc.tensor.matmul(
                    pso, lhsT=ones_row, rhs=bias4,
                    start=False, stop=True, skip_group_check=True,
                )
                ot = outs.tile([P, 4, OUT], f32, name="ot", tag="ot")
                for j in range(4):
                    src = pso[:, j * OUT : (j + 1) * OUT]
                    if (half + j) % 2 == 0:
                        nc.vector.tensor_scalar_max(ot[:, j, :], src, 0.0)
                    else:
                        nc.scalar.activation(ot[:, j, :], src, Act.Relu)
                nc.sync.dma_start(
                    out=o_out[g, half * 512 : (half + 1) * 512, :].rearrange(
                        "(j p) o -> p j o", p=P
                    ),
                    in_=ot,
                )

        for g in range(GP):
            load_phase(g)
            rest_phase(g)

    nc.finalize()
    return nc


def _get_nc():
    if "nc" not in _cached:
        _cached["nc"] = _build_nc()
    return _cached["nc"]


def kernel(X, A, kernel, bias):
    from concourse.bass_utils import run_bass_kernel_spmd

    nc = _get_nc()
    wk = np.ascontiguousarray(np.asarray(kernel, dtype=np.float32))
    bs = np.ascontiguousarray(np.asarray(bias, dtype=np.float32))
    A = np.asarray(A, dtype=np.float32)
    X = np.asarray(X, dtype=np.float32)
    in_maps = [
        {
            "a": np.ascontiguousarray(A[GP * c : GP * (c + 1)]),
            "x": np.ascontiguousarray(X[GP * c : GP * (c + 1)]),
            "wk": wk,
            "bias": bs,
        }
        for c in range(NCORES)
    ]
    res = run_bass_kernel_spmd(nc, in_maps, core_ids=list(range(NCORES)))
    return np.concatenate([r["out"] for r in res.results], axis=0)
